# revision 25
# baseline (speedup 1.0000x reference)
"""Distributed Bass kernel: 3D windowed attention with decomposed rel-pos bias.

Sharding: 8 cores = 4 batches x 2 head-groups (6 heads each).
Per-core layout is fully transposed ([channel, token]); the rel-pos bias is
folded into the scores matmul as 36 extra contraction channels (one-hot
k-position rows in the stationary operand, F = q.R tables in the moving
operand).  Softmax runs max-free (scores are O(+-8)); the denominator comes
free as a ones-row appended to V in the AV matmul.  All matmuls run in bf16
with fp32 PSUM accumulation.
"""

import os
import sys

import numpy as np

sys.path.insert(0, "/opt/trn_rl_repo")

B, D, H, W, C = 4, 8, 14, 14, 768
NH, HD = 12, 64
N = D * H * W  # 1568
HPC = 6  # heads per core
SCALE = HD ** -0.5
NKC = C // 128  # 6 k-chunks of input channels
NKT = (N + 127) // 128  # 13 token tiles (12x128 + 32)
FCH = [(0, 512), (512, 512), (1024, 512), (1536, 32)]  # free-dim chunks of N
KAUG = 100  # 64 qk channels + 8 + 14 + 14 bias channels

_CACHED = {}


def _build_nc():
    import concourse.bass as bass  # noqa: F401
    import concourse.mybir as mybir
    import concourse.tile as tile
    from concourse import bacc

    f32 = mybir.dt.float32
    bf16 = mybir.dt.bfloat16
    AF = mybir.ActivationFunctionType

    dbg = bool(int(os.environ.get("KERNEL_DEBUG", "0")))
    nc = bacc.Bacc(None, target_bir_lowering=False)

    # --- DRAM parameters (per-core shards; host pre-transposes/reorders) ---
    xT_d = nc.declare_dram_parameter("xT", [NKC, 128, N], bf16, isOutput=False)
    wqkv_d = nc.declare_dram_parameter("wqkv", [NKC, 128, 1152], bf16, isOutput=False)
    wproj_d = nc.declare_dram_parameter("wproj", [3, 128, 768], bf16, isOutput=False)
    oneh_d = nc.declare_dram_parameter("oneh", [36, N], bf16, isOutput=False)
    rdT_d = nc.declare_dram_parameter("rdT", [64, D * D], bf16, isOutput=False)
    rhT_d = nc.declare_dram_parameter("rhT", [64, H * H], bf16, isOutput=False)
    rwT_d = nc.declare_dram_parameter("rwT", [64, W * W], bf16, isOutput=False)
    bqk_d = nc.declare_dram_parameter("bqk", [128, 6], f32, isOutput=False)
    bv_d = nc.declare_dram_parameter("bv", [64, 6], f32, isOutput=False)
    out_d = nc.declare_dram_parameter("out", [NKC, 128, N], f32, isOutput=True)
    if dbg:
        dq_d = nc.declare_dram_parameter("dbg_q", [KAUG, N], bf16, isOutput=True)
        dk_d = nc.declare_dram_parameter("dbg_k", [KAUG, N], bf16, isOutput=True)
        dv_d = nc.declare_dram_parameter("dbg_v", [128, NKT * HPC * 65], bf16, isOutput=True)
        de_d = nc.declare_dram_parameter("dbg_e", [128, N], bf16, isOutput=True)
        dr_d = nc.declare_dram_parameter("dbg_r", [128, N], f32, isOutput=True)
        da_d = nc.declare_dram_parameter("dbg_a", [128, N], bf16, isOutput=True)

    with tile.TileContext(nc) as tc:
        with (
            tc.tile_pool(name="const", bufs=1) as cpool,
            tc.tile_pool(name="work", bufs=2) as wpool,
            tc.tile_pool(name="psum", bufs=4, space="PSUM") as mmp,
            tc.tile_pool(name="psav", bufs=1, space="PSUM") as avp,
        ):
            # ---- load constants ----
            xT = cpool.tile([128, NKC * N], bf16)
            wqkv = cpool.tile([128, NKC * 1152], bf16)
            wproj = cpool.tile([128, 3 * 768], bf16)
            oneh = cpool.tile([36, N], bf16)
            rdT = cpool.tile([64, D * D], bf16)
            rhT = cpool.tile([64, H * H], bf16)
            rwT = cpool.tile([64, W * W], bf16)
            bqk = cpool.tile([128, 6], f32)
            bv = cpool.tile([64, 6], f32)
            for kc in range(NKC):
                nc.sync.dma_start(xT[:, kc * N:(kc + 1) * N], xT_d[kc])
                nc.sync.dma_start(wqkv[:, kc * 1152:(kc + 1) * 1152], wqkv_d[kc])
            for t3 in range(3):
                nc.sync.dma_start(wproj[:, t3 * 768:(t3 + 1) * 768], wproj_d[t3])
            nc.sync.dma_start(oneh[:], oneh_d[:])
            nc.sync.dma_start(rdT[:], rdT_d[:])
            nc.sync.dma_start(rhT[:], rhT_d[:])
            nc.sync.dma_start(rwT[:], rwT_d[:])
            nc.sync.dma_start(bqk[:], bqk_d[:])
            nc.sync.dma_start(bv[:], bv_d[:])

            # ---- V in natural [token, channel] layout, ones column per head ----
            vnat = cpool.tile([128, NKT, HPC * 65], bf16)
            nc.vector.memset(vnat[:], 1.0)
            for kt in range(NKT):
                kp = min(128, N - kt * 128)
                pv = mmp.tile([128, 512], f32, tag="mm")
                for kc in range(NKC):
                    nc.tensor.matmul(
                        pv[0:kp, 0:384],
                        xT[:, kc * N + kt * 128: kc * N + kt * 128 + kp],
                        wqkv[:, kc * 1152 + 768: kc * 1152 + 1152],
                        start=(kc == 0), stop=(kc == NKC - 1),
                    )
                for h6 in range(HPC):
                    nc.vector.tensor_copy(
                        vnat[0:kp, kt, h6 * 65:h6 * 65 + 64],
                        pv[0:kp, h6 * 64:(h6 + 1) * 64],
                    )

            av_all = [
                cpool.tile([128, N], bf16, name=f"av_all{i}", tag=f"av{i}")
                for i in range(3)
            ]

            # ---- head pairs ----
            for p in range(3):
                augs = []
                for x in range(2):
                    q_t = wpool.tile([KAUG, N], bf16, tag=f"qaug{x}")
                    k_t = wpool.tile([128, N], bf16, tag=f"kaug{x}")
                    augs.append((q_t, k_t))
                # q then k projections for the pair (M=128 covers both heads)
                for qk in range(2):
                    col0 = qk * 384 + p * 128
                    bcol = qk * 3 + p
                    for (f0, fl) in FCH:
                        ps = mmp.tile([128, 512], f32, tag="mm")
                        for kc in range(NKC):
                            nc.tensor.matmul(
                                ps[:, 0:fl],
                                wqkv[:, kc * 1152 + col0: kc * 1152 + col0 + 128],
                                xT[:, kc * N + f0: kc * N + f0 + fl],
                                start=(kc == 0), stop=(kc == NKC - 1),
                            )
                        for x in range(2):
                            dst = augs[x][qk]
                            nc.vector.tensor_scalar_add(
                                dst[0:64, f0:f0 + fl],
                                ps[x * 64:(x + 1) * 64, 0:fl],
                                bqk[x * 64:(x + 1) * 64, bcol:bcol + 1],
                            )
                for x in range(2):
                    q_t, k_t = augs[x]
                    nc.vector.tensor_copy(k_t[64:KAUG, :], oneh[:])
                    qv = q_t[0:64, :].rearrange("p (d h w) -> p d h w", d=D, h=H, w=W)
                    for qd in range(D):
                        pf = mmp.tile([128, 512], f32, tag="mm")
                        nc.tensor.matmul(
                            pf[0:D, 0:H * W],
                            rdT[:, qd * D:(qd + 1) * D],
                            q_t[0:64, qd * H * W:(qd + 1) * H * W],
                        )
                        nc.vector.tensor_copy(
                            q_t[64:72, qd * H * W:(qd + 1) * H * W], pf[0:D, 0:H * W]
                        )
                    # F_h / F_w land at non-32-aligned partitions of q_t, which
                    # engines cannot write; stage at base 0 and DMA-shift.
                    fsh = wpool.tile([14, N], bf16, tag="fsh")
                    fsw = wpool.tile([14, N], bf16, tag="fsw")
                    sh = fsh.rearrange("p (d h w) -> p d h w", d=D, h=H, w=W)
                    sw = fsw.rearrange("p (d h w) -> p d h w", d=D, h=H, w=W)
                    for qh in range(H):
                        pf = mmp.tile([128, 512], f32, tag="mm")
                        nc.tensor.matmul(
                            pf[0:H, 0:D * W], rhT[:, qh * H:(qh + 1) * H], qv[:, :, qh, :]
                        )
                        nc.vector.tensor_copy(sh[:, :, qh, :], pf[0:H, 0:D * W])
                    for qw in range(W):
                        pf = mmp.tile([128, 512], f32, tag="mm")
                        nc.tensor.matmul(
                            pf[0:W, 0:D * H], rwT[:, qw * W:(qw + 1) * W], qv[:, :, :, qw]
                        )
                        nc.vector.tensor_copy(sw[:, :, :, qw], pf[0:W, 0:D * H])
                    nc.sync.dma_start(q_t[72:86, :], fsh[:])
                    nc.sync.dma_start(q_t[86:100, :], fsw[:])
                    if dbg and p == 0 and x == 0:
                        nc.sync.dma_start(dq_d[:], q_t[0:KAUG, :])
                        nc.sync.dma_start(dk_d[:], k_t[0:KAUG, :])
                        nc.sync.dma_start(dv_d[:], vnat[:].rearrange("p a b -> p (a b)"))
                # attention per head of the pair
                for x in range(2):
                    h6 = 2 * p + x
                    q_t, k_t = augs[x]
                    pav = avp.tile([65, N], f32, tag="av")
                    for kt in range(NKT):
                        kp = min(128, N - kt * 128)
                        et = wpool.tile([128, N], bf16, tag="exp")
                        for (f0, fl) in FCH:
                            ps = mmp.tile([128, 512], f32, tag="mm")
                            nc.tensor.matmul(
                                ps[0:kp, 0:fl],
                                k_t[0:KAUG, kt * 128: kt * 128 + kp],
                                q_t[0:KAUG, f0:f0 + fl],
                            )
                            nc.scalar.activation(
                                et[0:kp, f0:f0 + fl], ps[0:kp, 0:fl], AF.Exp
                            )
                        if dbg and h6 == 0 and kt == 0:
                            nc.sync.dma_start(de_d[:], et[:])
                        for (f0, fl) in FCH:
                            nc.tensor.matmul(
                                pav[:, f0:f0 + fl],
                                vnat[0:kp, kt, h6 * 65:(h6 + 1) * 65],
                                et[0:kp, f0:f0 + fl],
                                start=(kt == 0), stop=(kt == NKT - 1),
                            )
                    recip = wpool.tile([1, N], f32, tag="recip")
                    rbc = wpool.tile([64, N], f32, tag="rbc")
                    nc.vector.reciprocal(recip[0:1, :], pav[64:65, :])
                    nc.gpsimd.partition_broadcast(rbc[0:64, :], recip[0:1, :])
                    avrows = av_all[p][x * 64:(x + 1) * 64, :]
                    nc.vector.tensor_mul(avrows, pav[0:64, :], rbc[0:64, :])
                    nc.scalar.activation(
                        avrows, avrows, AF.Identity, bias=bv[:, h6:h6 + 1]
                    )
                    if dbg and h6 == 0:
                        nc.sync.dma_start(dr_d[:], rbc[:])
                    if dbg and p == 0 and x == 1:
                        nc.sync.dma_start(da_d[:], av_all[0][:])

            # ---- partial projection: outT[768, N] ----
            for mo in range(NKC):
                ot = wpool.tile([128, N], f32, tag="out")
                for (f0, fl) in FCH:
                    ps = mmp.tile([128, 512], f32, tag="mm")
                    for t3 in range(3):
                        nc.tensor.matmul(
                            ps[:, 0:fl],
                            wproj[:, t3 * 768 + mo * 128: t3 * 768 + mo * 128 + 128],
                            av_all[t3][:, f0:f0 + fl],
                            start=(t3 == 0), stop=(t3 == 2),
                        )
                    nc.vector.tensor_copy(ot[:, f0:f0 + fl], ps[:, 0:fl])
                nc.sync.dma_start(out_d[mo], ot[:])

    nc.compile()
    return nc


def _prep_inputs(x, qkv_w, qkv_b, proj_w, proj_b, rel_pos_d, rel_pos_h, rel_pos_w):
    """Host-side shard prep: returns in_maps list for 8 cores."""
    import ml_dtypes
    bf = ml_dtypes.bfloat16
    x = np.ascontiguousarray(x, np.float32)
    qkv_w = np.asarray(qkv_w, np.float32)
    qkv_b = np.asarray(qkv_b, np.float32)
    proj_w = np.asarray(proj_w, np.float32)

    # one-hot k-position rows [36, N]
    j = np.arange(N)
    kd, kh, kw = j // (H * W), (j // W) % H, j % W
    oneh = np.zeros((36, N), np.float32)
    oneh[kd, j] = 1.0
    oneh[8 + kh, j] = 1.0
    oneh[22 + kw, j] = 1.0
    oneh = oneh.astype(bf)

    # rel tables, transposed and un-scaled (q is pre-scaled by SCALE)
    def rtab(table, n):
        t = np.asarray(table, np.float32) / SCALE  # [2n-1, 64]
        qq, kk = np.meshgrid(np.arange(n), np.arange(n), indexing="ij")
        return np.ascontiguousarray(
            t[(qq - kk + n - 1).reshape(-1)].T
        ).astype(bf)  # [64, n*n], col = q*n+k

    rdT = rtab(rel_pos_d, D)
    rhT = rtab(rel_pos_h, H)
    rwT = rtab(rel_pos_w, W)

    in_maps = []
    for core in range(8):
        b, g = divmod(core, 2)
        heads = list(range(g * HPC, (g + 1) * HPC))
        # W columns: [q(6x64) | k(6x64) | v(6x64)] for this head group; q scaled
        cols_q = [0 * C + h * HD + c for h in heads for c in range(HD)]
        cols_k = [1 * C + h * HD + c for h in heads for c in range(HD)]
        cols_v = [2 * C + h * HD + c for h in heads for c in range(HD)]
        wq = qkv_w[:, cols_q] * SCALE
        wk = qkv_w[:, cols_k]
        wv = qkv_w[:, cols_v]
        wc = np.concatenate([wq, wk, wv], axis=1)  # [768, 1152]
        wqkv = np.ascontiguousarray(wc.reshape(NKC, 128, 1152)).astype(bf)

        bq = qkv_b[cols_q] * SCALE
        bk = qkv_b[cols_k]
        bvv = qkv_b[cols_v]
        bqk = np.zeros((128, 6), np.float32)
        for p in range(3):
            bqk[:, p] = bq[p * 128:(p + 1) * 128]
            bqk[:, 3 + p] = bk[p * 128:(p + 1) * 128]
        bv_t = np.ascontiguousarray(bvv.reshape(6, 64).T, np.float32)  # [64,6]

        rows = [h * HD + c for h in heads for c in range(HD)]
        wp = np.ascontiguousarray(proj_w[rows].reshape(3, 128, 768)).astype(bf)

        xT = np.ascontiguousarray(
            x[b].reshape(N, C).T.reshape(NKC, 128, N)
        ).astype(bf)
        in_maps.append({
            "xT": xT, "wqkv": wqkv, "wproj": wp, "oneh": oneh,
            "rdT": rdT, "rhT": rhT, "rwT": rwT, "bqk": bqk, "bv": bv_t,
        })
    return in_maps


def _install_ntff_hook_shim():
    """The image's antenv package lacks axon_hooks; recreate it so
    run_bass_kernel_spmd(trace=True) can reach the libaxon NTFF profiler."""
    import types

    if "antenv.axon_hooks" in sys.modules:
        return
    import antenv
    mod = types.ModuleType("antenv.axon_hooks")
    _hook = [None]
    mod.set_axon_ntff_profile_hook = lambda h: _hook.__setitem__(0, h)
    mod.get_axon_ntff_profile_hook = lambda: _hook[0]
    antenv.axon_hooks = mod
    sys.modules["antenv.axon_hooks"] = mod
    try:
        from trn_agent_boot.trn_boot import _ntff_profile_via_ctypes

        mod.set_axon_ntff_profile_hook(
            _ntff_profile_via_ctypes("/opt/axon/libaxon_pjrt.so")
        )
    except Exception as e:  # degrade to no tracing
        print(f"ntff hook shim failed: {e}", file=sys.stderr)


LAST_EXEC_NS = None


def kernel(x, qkv_w, qkv_b, proj_w, proj_b, rel_pos_d, rel_pos_h, rel_pos_w):
    global LAST_EXEC_NS
    if "nc" not in _CACHED:
        _CACHED["nc"] = _build_nc()
    nc = _CACHED["nc"]
    in_maps = _prep_inputs(
        x, qkv_w, qkv_b, proj_w, proj_b, rel_pos_d, rel_pos_h, rel_pos_w
    )
    from concourse.bass_utils import run_bass_kernel_spmd

    trace = bool(int(os.environ.get("KERNEL_TRACE", "0")))
    if trace:
        _install_ntff_hook_shim()
    res = run_bass_kernel_spmd(nc, in_maps, core_ids=list(range(8)), trace=trace)
    LAST_EXEC_NS = res.exec_time_ns
    proj_b = np.asarray(proj_b, np.float32)
    outs = []
    for b in range(B):
        t0 = res.results[2 * b]["out"].reshape(C, N)
        t1 = res.results[2 * b + 1]["out"].reshape(C, N)
        outs.append((t0 + t1).T + proj_b)
    return np.stack(outs).reshape(B, D, H, W, C).astype(np.float32)
